# revision 1
# baseline (speedup 1.0000x reference)
"""Trainium2 Bass kernel for AttentionOptimizedNeuron (v2 — transpose-free).

Model (per channel c=(b,d), recurrence over t):
    att = A_ATT*att + aw[t]*GAIN
    mem = A_MEM*mem + x[t]*(1+att)
    s   = (mem >= 1);  mem -= s          (subtract reset)

Key ideas vs v1:
  - All layout work happens on the HOST: inputs arrive channel-major
    [128c, 32pg, T] with x pre-scaled by alpha_mem^-t (global rescale);
    outputs leave channel-major as bf16 and are transposed back on host.
    The device does ZERO transposes (v1's DMA-xbar transposes dominated
    its 13 ms runtime).
  - The global rescale turns the membrane recurrence into a pure add-scan:
        UH_t = sum_{j<=t} A_j*(1+qg_j),  A_j = x_j*alpha_mem^-j
    computed by ONE fused custom-DVE scan op per page (kills the separate
    p = x*(1+att) and c = (uh-1)*alpha^-tau bulk passes of v1).
  - qg = GAIN*att via the native affine scan (GpSimd rejects ALU ops in
    this toolchain, so all elementwise work rides the DVE).
  - Spike recurrence in globally rescaled debt space:
        s_t = (V <= UH_t - alpha^-t),  V += alpha^-t * s_t
    one fused custom-DVE instruction per (t, half-page group), two
    interleaved chains so drains overlap; the compare reads UH directly.
    Spikes recovered afterwards as is_ne(V', V) (bf16 out).
  - Chunking (L=128) is memory-only: the add-scan restarts at 0 each chunk
    and the chunk's final UH is subtracted from V (exact identity), so no
    per-page carry state is needed in the custom scan.
"""

import numpy as np

A_MEM = float(np.exp(-1.0 / 20.0))
A_ATT = float(np.exp(-1.0 / 50.0))
GAIN = 0.2

B, T, D = 32, 1024, 1024
NCORES = 8
BPC = B // NCORES            # batches per core
NPAGES = BPC * (D // 128)    # 32 channel pages of 128
L = 128                      # time steps per chunk (SBUF sizing)
NCHUNK = T // L
H = NPAGES // 2              # half-page group for interleaved W chains
W_SINGLE = False
HOST_S = True

_STATE = {}


def _split_waits(nc):
    """walrus CoreV3 in this container rejects >1 sync wait per instruction.
    Tile attaches several; move the extras onto same-engine nops inserted
    immediately before the instruction (identical blocking semantics)."""
    from concourse import mybir

    for f in nc.m.functions:
        for blk in f.blocks:
            new_insts = []
            for inst in blk.instructions:
                si = getattr(inst, "sync_info", None)
                if si is not None and si.on_wait and len(si.on_wait) > 1:
                    waits = list(si.on_wait)
                    si.on_wait = waits[-1:]
                    for w in waits[:-1]:
                        nop = mybir.InstNoOp(
                            name=nc.get_next_instruction_name(),
                            opcode="NoOp",
                            engine=inst.engine,
                            sync_info=mybir.SyncInfo(on_wait=[w], on_update=[]),
                        )
                        new_insts.append(nop)
                new_insts.append(inst)
            if len(new_insts) != len(blk.instructions):
                blk.instructions[:] = new_insts


def _strip_chain_waits(nc, decr_names):
    """The W loop alternates two independent half-page chains; each
    instruction's true producer is two DVE instructions back, but Tile's
    self-sem wait targets one back (the other chain's drain), which would
    serialize the chains. Waiting for value-1 is the true dependency and
    is already satisfied at dispatch."""
    for f in nc.m.functions:
        for blk in f.blocks:
            for inst in blk.instructions:
                if (getattr(inst, "isa_opcode", None) in (0xAE, 0xAF, 0xEE, 0xEF)
                        and inst.name in decr_names):
                    si = inst.sync_info
                    if si is not None and si.on_wait:
                        for w in si.on_wait:
                            if str(w.ant_name).startswith("DVE") and w.wait_value > 0:
                                w.wait_value = w.wait_value - 1


def _patch_sim_visit():
    import concourse.bass_interp as bi
    if hasattr(bi, "_orig_visit_instisa"):
        return
    bi._orig_visit_instisa = bi._visit_InstISA

    def _pv(isa, instruction, core_sim):
        if instruction.isa_opcode in (0xAE, 0xAF, 0xEE, 0xEF):
            return
        return bi._orig_visit_instisa(isa, instruction, core_sim)

    bi._visit_InstISA = _pv


def _lower_segmented(spec, ver):
    """lower() for a single-scan Spec, with a per-subdim STEP state so the
    scan accumulator re-seeds at each SUB_DIM boundary (page) — the 3-state
    machine of 05-custom-dve-design.md (SEED / STEADY / STEP)."""
    import dataclasses
    from concourse import dve_spec as ds
    from concourse.dve_uop import Trigger

    ds._validate_body(spec, ver)
    spec2 = ds._hoist_stream_invariant_ops(spec)
    scans = ds._collect(spec2.body, ds.Scan)
    latches = ds._collect(spec2.body, ds.Latch)
    assert len(scans) == 1 and not latches
    p = ds._build_placement(spec2, scans, ds.N_STAGES[ver], ds.N_LANES[ver])
    states = ds._build_state_machine(spec2, scans, latches, p)
    assert len(states) == 2, "expected [seed, steady]"
    seed, steady = states
    sc = scans[0]
    d = p.node_stage[sc]
    # STEP: acc = op(identity, expr) == expr  (ADD-scan re-seed to 0)
    if isinstance(sc.expr, ds.Alu):
        ov = ds._Stage(ds.AluOp.BYPASS, ds.PREV)
    else:
        ov = ds._Stage(ds.AluOp.BYPASS, sc.expr)
    steady2 = dataclasses.replace(
        steady,
        trigger=(Trigger.SRC_TENSOR_DONE, Trigger.SUB_DIM_DONE, Trigger.NONE),
        next=(0, 2, 0))
    step = dataclasses.replace(
        steady,
        overrides={**steady.overrides, d: ov},
        trigger=(Trigger.SRC_TENSOR_DONE, Trigger.SUB_DIM_DONE, Trigger.COUNT),
        next=(0, 2, 1), repeat=1)
    out = [ds._assemble(s) for s in (seed, steady2, step)]
    for u in out:
        u.validate(ver)
    return out


def _register_ops():
    from concourse import dve_ops
    from concourse.dve_ops import DveOp
    from concourse.dve_spec import Spec, Src0, Src1, C0, One, scan, AluOp, lower
    from concourse.dve_uop import DveOpSpec

    def reg(name, spec, segmented=False):
        for op in dve_ops.OPS:
            if op.name == name:
                return
        row = max(dve_ops._SUB_OPCODE_FOR_NAME.values()) + 1
        assert row < 0x20
        dve_ops._SUB_OPCODE_FOR_NAME[name] = row
        lower_fn = _lower_segmented if segmented else lower
        shas = {}
        specs = {}
        for ver in ("v3", "v4"):
            s = DveOpSpec(name=name, opcode=row, uops=lower_fn(spec, ver=ver)
                          if not segmented else _lower_segmented(spec, ver),
                          rd1_en=True)
            shas[ver] = s.sha(ver)
            specs[ver] = s
        op = DveOp(name, spec, subdim=bool(segmented), uops_sha=shas)
        dve_ops.OPS.append(op)
        dve_ops.CUSTOM_DVE_SPECS[name] = spec
        if segmented:
            # op.compile() would re-lower via the plain path; pre-seed its
            # cache with the segmented programs instead.
            for ver in ("v3", "v4"):
                dve_ops._COMPILE_CACHE[(name, ver)] = specs[ver]

    reg("UHSCAN_ANT", Spec(
        body=scan(AluOp.ADD, Src0 * (One + Src1)),
        reference=lambda in0, in1, s0, s1, imm2:
            np.cumsum(in0 * (1.0 + in1), axis=-1, dtype=np.float32),
    ))
    reg("QGSCAN_ANT", Spec(
        body=scan(AluOp.ADD, Src0) * Src1,
        reference=lambda in0, in1, s0, s1, imm2:
            (np.cumsum(in0, axis=-1, dtype=np.float32) * in1).astype(np.float32),
    ))
    def _ref_uhseg(in0, in1, s0, s1, imm2):
        c = np.cumsum((in0 * (1.0 + in1)).reshape(in0.shape[0], NPAGES, L),
                      axis=-1, dtype=np.float32)
        return c.reshape(in0.shape)

    def _ref_qgseg(in0, in1, s0, s1, imm2):
        c = np.cumsum(in0.reshape(in0.shape[0], NPAGES, L), axis=-1,
                      dtype=np.float32).reshape(in0.shape)
        return (c * in1).astype(np.float32)

    reg("UHSEG_ANT", Spec(
        body=scan(AluOp.ADD, Src0 * (One + Src1)),
        reference=_ref_uhseg,
    ), segmented=True)
    reg("QGSEG_ANT", Spec(
        body=scan(AluOp.ADD, Src0) * Src1,
        reference=_ref_qgseg,
    ), segmented=True)
    reg("WSTEP2_ANT", Spec(
        body=Src0 + ((Src0 <= (Src1 - C0)) * C0),
        reference=lambda in0, in1, s0, s1, imm2:
            in0 + (in0 <= in1 - s0).astype(np.float32) * s0,
    ))


def _build():
    from contextlib import ExitStack
    import concourse.bass as bass
    import concourse.mybir as mybir
    from concourse.tile import TileContext
    from concourse.dve_ops import get_dve_sub_opcode

    f32 = mybir.dt.float32
    bf16 = mybir.dt.bfloat16
    Alu = mybir.AluOpType

    _patch_sim_visit()
    _register_ops()
    ROW_UH = get_dve_sub_opcode("UHSCAN_ANT")
    ROW_QG = get_dve_sub_opcode("QGSCAN_ANT")
    ROW_UHSEG = get_dve_sub_opcode("UHSEG_ANT")
    ROW_QGSEG = get_dve_sub_opcode("QGSEG_ANT")
    SEG = True
    ROW_W = get_dve_sub_opcode("WSTEP2_ANT")

    nc = bass.Bass()
    a_in = nc.dram_tensor("a", (128, NPAGES, T), f32, kind="ExternalInput")
    b_in = nc.dram_tensor("b", (128, NPAGES, T), f32, kind="ExternalInput")
    if HOST_S:
        s_out = nc.dram_tensor("s", (128, NCHUNK, L + 1, NPAGES), f32,
                               kind="ExternalOutput")
    else:
        s_out = nc.dram_tensor("s", (128, NCHUNK, L, NPAGES), bf16,
                               kind="ExternalOutput")

    es = ExitStack()
    # static double-buffered working set (custom-ISA structs need
    # trace-time addresses, so no tile pools here)
    At = es.enter_context(nc.sbuf_tensor([128, 2, NPAGES, L], f32))
    Bt = es.enter_context(nc.sbuf_tensor([128, 2, NPAGES, L], f32))
    qg = es.enter_context(nc.sbuf_tensor([128, 2, NPAGES, L], f32))
    uh = es.enter_context(nc.sbuf_tensor([128, 2, L, NPAGES], f32))  # tau-major
    w = es.enter_context(nc.sbuf_tensor([128, 2, L + 1, NPAGES], f32))
    sb = es.enter_context(nc.sbuf_tensor([128, 2, L, NPAGES], bf16))
    rtile = es.enter_context(nc.sbuf_tensor([128, L], f32))
    rtile32 = es.enter_context(nc.sbuf_tensor([128, NPAGES, L], f32))
    rstage = es.enter_context(nc.sbuf_tensor([128, L], f32))
    qcar = es.enter_context(nc.sbuf_tensor([128, NPAGES], f32))

    mls_addr = {}
    for f in nc.m.functions:
        for a in f.allocations:
            if isinstance(a, mybir.MemoryLocationSet) and a.memorylocations:
                mls_addr[a.memorylocations[0].name] = a.memorylocations[0].addr

    def addr_of(ap):
        return mls_addr[ap.tensor.alloc_name] + ap.offset * mybir.dt.size(ap.dtype)

    def emit_custom(row, out_ap, out_sn, in0_ap, in0_sn, in1_ap, in1_sn, imm0,
                    op1=0, pat=None):
        """(step, num) given explicitly for each operand's free pattern."""
        def _p2(ap, sn):
            if len(sn) == 4:
                return {"start_addr": {"addr_immediate": addr_of(ap)},
                        "step_elem": [sn[0], sn[2]], "num_elem": [sn[1], sn[3]]}
            return {"start_addr": {"addr_immediate": addr_of(ap)},
                    "step_elem": [sn[0], 0], "num_elem": [sn[1], 1]}

        struct = {
            "src0_mem_pattern": _p2(in0_ap, in0_sn),
            "src1_mem_pattern": {
                "start_addr": {"addr_immediate": addr_of(in1_ap)},
                "step_elem": [in1_sn[0]], "num_elem": [in1_sn[1]]},
            "dst_mem_pattern": _p2(out_ap, out_sn),
            "in0_in1_dtype": {"dtype_lo": 10, "dtype_hi": 10},
            "out_dtype": 10,
            "num_active_channels": 128,
            "imm0_src": 0, "imm1_src": 0, "imm2_src": 1,
            "imm0": {"imm_arith_fp32": float(imm0)},
            "imm1": {"imm_arith_fp32": 0.0},
            "imm2": {"imm_arith_fp32": 0.0},
            "op0": row | (1 << 5),
            "op1": op1,
        }
        return nc.vector.isa(
            nc.isa.Opcode.NEURON_ISA_TPB_OPCODE_CUSTOM_DVE_ANT_0, struct,
            ins=[nc.vector.lower_ap(in0_ap), nc.vector.lower_ap(in1_ap)],
            outs=[nc.vector.lower_ap(out_ap)],
        )

    decr = set()
    with TileContext(nc) as tc:
        # rtile[tau] = A_ATT**tau (scan: r' = A_ATT*r, seeded by 1/A_ATT)
        nc.vector.memset(rstage[:], A_ATT)
        nc.vector.tensor_tensor_scan(
            rtile[:], rstage[:], rstage[:], float(1.0 / A_ATT),
            op0=Alu.mult, op1=Alu.bypass)
        for pg in range(NPAGES):
            nc.vector.tensor_copy(rtile32[:, pg, :], rtile[:])
        nc.vector.memset(qcar[:], 0.0)
        for ci in range(NCHUNK):
            t0 = ci * L
            k = ci % 2
            kp = (ci - 1) % 2
            nc.sync.dma_start(At[:, k], a_in.ap()[:, :, t0:t0 + L])
            nc.scalar.dma_start(Bt[:, k], b_in.ap()[:, :, t0:t0 + L])

            # attention trace (GAIN-scaled): qg_tau = A_ATT^tau * (kappa +
            # sum_{j<=tau} B_j), B host-prescaled by GAIN*A_ATT^-tau; the
            # cross-chunk carry kappa is injected into the first stream element
            nc.vector.tensor_tensor(
                out=Bt[:, k, :, 0], in0=Bt[:, k, :, 0], in1=qcar[:],
                op=Alu.add)
            if SEG:
                emit_custom(ROW_QGSEG,
                            qg[:, k], (1, L, L, NPAGES),
                            Bt[:, k], (1, L, L, NPAGES),
                            rtile32[:], (1, L * NPAGES), 0.0,
                            op1=0x02)
            else:
                for pg in range(NPAGES):
                    emit_custom(ROW_QG,
                                qg[:, k, pg, :], (1, L),
                                Bt[:, k, pg, :], (1, L),
                                rtile[:, :], (1, L), 0.0)
            nc.vector.tensor_scalar(
                out=qcar[:], in0=qg[:, k, :, L - 1], scalar1=float(A_ATT),
                scalar2=None, op0=Alu.mult)

            # chunk-local rescaled membrane: UH = cumsum(A*(1+qg))
            if SEG:
                emit_custom(ROW_UHSEG,
                            uh[:, k], (NPAGES, L, 1, NPAGES),
                            At[:, k], (1, L, L, NPAGES),
                            qg[:, k], (1, L * NPAGES), 0.0,
                            op1=0x02)
            else:
                for pg in range(NPAGES):
                    emit_custom(ROW_UH,
                                uh[:, k, :, pg], (NPAGES, L),
                                At[:, k, pg, :], (1, L),
                                qg[:, k, pg, :], (1, L), 0.0)

            # spike recurrence in rescaled debt space
            if ci == 0:
                nc.vector.memset(w[:, k, 0, :], 0.0)
            else:
                nc.vector.tensor_tensor(
                    out=w[:, k, 0, :], in0=w[:, kp, L, :],
                    in1=uh[:, kp, L - 1, :], op=Alu.subtract)
            for tau in range(L):
                aa = float(A_MEM ** (-(t0 + tau)))
                if W_SINGLE:
                    ia = emit_custom(ROW_W,
                                     w[:, k, tau + 1, :], (1, NPAGES),
                                     w[:, k, tau, :], (1, NPAGES),
                                     uh[:, k, tau, :], (1, NPAGES), aa)
                    if tau >= 1:
                        decr.add(ia.ins.name)
                else:
                    ia = emit_custom(ROW_W,
                                     w[:, k, tau + 1, 0:H], (1, H),
                                     w[:, k, tau, 0:H], (1, H),
                                     uh[:, k, tau, 0:H], (1, H), aa)
                    ib = emit_custom(ROW_W,
                                     w[:, k, tau + 1, H:NPAGES], (1, H),
                                     w[:, k, tau, H:NPAGES], (1, H),
                                     uh[:, k, tau, H:NPAGES], (1, H), aa)
                    if tau >= 1:
                        decr.add(ia.ins.name)
                        decr.add(ib.ins.name)

            # spikes: V moved iff spike fired
            if HOST_S:
                nc.scalar.dma_start(s_out.ap()[:, ci], w[:, k])
            else:
                nc.vector.tensor_tensor(
                    out=sb[:, k], in0=w[:, k, 1:L + 1, :], in1=w[:, k, 0:L, :],
                    op=Alu.not_equal)
                nc.scalar.dma_start(s_out.ap()[:, ci], sb[:, k])
    es.close()
    nc.m.ant_custom_dve_ops = sorted(
        {*nc.m.ant_custom_dve_ops, "UHSCAN_ANT", "QGSCAN_ANT", "WSTEP2_ANT", "UHSEG_ANT", "QGSEG_ANT"})
    _strip_chain_waits(nc, decr)
    _split_waits(nc)
    return nc


def kernel(x: np.ndarray, attention_weights: np.ndarray) -> np.ndarray:
    from concourse.bass_utils import run_bass_kernel_spmd

    if "nc" not in _STATE:
        _STATE["nc"] = _build()
    nc = _STATE["nc"]

    x = np.ascontiguousarray(x, dtype=np.float32)
    aw = np.ascontiguousarray(attention_weights, dtype=np.float32)

    # host layout + prescale: [b, t, j, c] -> [c, b, j, t]
    invm = np.exp(np.arange(T, dtype=np.float64) / 20.0).astype(np.float32)
    A_all = np.ascontiguousarray(
        x.reshape(B, T, D // 128, 128).transpose(3, 0, 2, 1))
    A_all *= invm[None, None, None, :]
    B_all = np.ascontiguousarray(
        aw.reshape(B, T, D // 128, 128).transpose(3, 0, 2, 1))
    invb = (GAIN * np.exp((np.arange(T, dtype=np.float64) % L) / 50.0)
            ).astype(np.float32)
    B_all *= invb[None, None, None, :]

    in_maps = [
        {"a": A_all[:, k * BPC:(k + 1) * BPC].reshape(128, NPAGES, T),
         "b": B_all[:, k * BPC:(k + 1) * BPC].reshape(128, NPAGES, T)}
        for k in range(NCORES)
    ]
    res = run_bass_kernel_spmd(nc, in_maps, core_ids=list(range(NCORES)))

    out = np.empty((B, T, D), dtype=np.float32)
    for k in range(NCORES):
        if HOST_S:
            wtr = np.asarray(res.results[k]["s"], dtype=np.float32)
            s = (wtr[:, :, 1:, :] != wtr[:, :, :-1, :]).astype(np.float32)
        else:
            s = np.asarray(res.results[k]["s"]).astype(np.float32)
        # [c, ci, tau, pg] -> [b, ci, tau, j, c] -> [b, t, d]
        s = s.reshape(128, NCHUNK, L, BPC, D // 128).transpose(3, 1, 2, 4, 0)
        out[k * BPC:(k + 1) * BPC] = s.reshape(BPC, T, D)
    return out



# revision 2
# speedup vs baseline: 1.4115x; 1.4115x over previous
"""Trainium2 Bass kernel for AttentionOptimizedNeuron (v3 — WRING scan).

Model (per channel c=(b,d), recurrence over t):
    att = A_ATT*att + aw[t]*GAIN
    mem = A_MEM*mem + x[t]*(1+att)
    s   = (mem >= 1);  mem -= s          (subtract reset)

v3 vs v2: the spike/reset recurrence runs as ONE hand-written custom-DVE
scan instruction per chunk (WRING_ANT) instead of 2048 tiny chained
instructions. In chunk-rescaled debt space (V = cumulative reset debt,
UH = chunk-local cumsum of the rescaled drive, C_tau = A_MEM^-tau):

    s_tau = (UH_tau - C_tau >= V);  V += C_tau * s_tau

The WRING uOp program closes this feedback loop on the DVE datapath:
stage0 d=UH-C, stage1 cmp=IS_GE(d, V via NEXT_ALU_OUT_A), stage2
gate=cmp*C, stage3 V+=gate (CURR feedback + a-flop). V is brigaded
backward (stage3.a -> stage2.a) by bubble slots; II=4 cycles/element.
Per-page V re-seed rides the stream: each page's position 0 holds the
carry (V_L - UH_L of the previous chunk), consumed by a STEP uop state
(V <- carry * alpha^L via the C-stream boundary slot), triggered by
SUB_DIM_DONE at page wrap. Spikes recovered on host as diff(V) != 0.

Host prescale is chunk-local (alpha^-(t mod L)), so magnitudes stay in
[1, 602] per chunk regardless of t.
"""

import numpy as np

A_MEM = float(np.exp(-1.0 / 20.0))
A_ATT = float(np.exp(-1.0 / 50.0))
GAIN = 0.2

B, T, D = 32, 1024, 1024
NCORES = 8
BPC = B // NCORES            # batches per core
NPAGES = BPC * (D // 128)    # 32 channel pages of 128
L = 128                      # time steps per chunk (SBUF sizing)
NCHUNK = T // L
L1 = L + 1                   # page stream length incl. carry slot

_STATE = {}


def _split_waits(nc):
    """walrus CoreV3 in this container rejects >1 sync wait per instruction.
    Tile attaches several; move the extras onto same-engine nops inserted
    immediately before the instruction (identical blocking semantics)."""
    from concourse import mybir

    for f in nc.m.functions:
        for blk in f.blocks:
            new_insts = []
            for inst in blk.instructions:
                si = getattr(inst, "sync_info", None)
                if si is not None and si.on_wait and len(si.on_wait) > 1:
                    waits = list(si.on_wait)
                    si.on_wait = waits[-1:]
                    for w in waits[:-1]:
                        nop = mybir.InstNoOp(
                            name=nc.get_next_instruction_name(),
                            opcode="NoOp",
                            engine=inst.engine,
                            sync_info=mybir.SyncInfo(on_wait=[w], on_update=[]),
                        )
                        new_insts.append(nop)
                new_insts.append(inst)
            if len(new_insts) != len(blk.instructions):
                blk.instructions[:] = new_insts


def _patch_sim_visit():
    import concourse.bass_interp as bi
    if hasattr(bi, "_orig_visit_instisa"):
        return
    bi._orig_visit_instisa = bi._visit_InstISA

    def _pv(isa, instruction, core_sim):
        if instruction.isa_opcode in (0xAE, 0xAF, 0xEE, 0xEF):
            return
        return bi._orig_visit_instisa(isa, instruction, core_sim)

    bi._visit_InstISA = _pv


def _lower_segmented(spec, ver):
    """lower() for a single-scan Spec, with a per-subdim STEP state so the
    scan accumulator re-seeds at each SUB_DIM boundary (page)."""
    import dataclasses
    from concourse import dve_spec as ds
    from concourse.dve_uop import Trigger

    ds._validate_body(spec, ver)
    spec2 = ds._hoist_stream_invariant_ops(spec)
    scans = ds._collect(spec2.body, ds.Scan)
    latches = ds._collect(spec2.body, ds.Latch)
    assert len(scans) == 1 and not latches
    p = ds._build_placement(spec2, scans, ds.N_STAGES[ver], ds.N_LANES[ver])
    states = ds._build_state_machine(spec2, scans, latches, p)
    assert len(states) == 2, "expected [seed, steady]"
    seed, steady = states
    sc = scans[0]
    d = p.node_stage[sc]
    if isinstance(sc.expr, ds.Alu):
        ov = ds._Stage(ds.AluOp.BYPASS, ds.PREV)
    else:
        ov = ds._Stage(ds.AluOp.BYPASS, sc.expr)
    steady2 = dataclasses.replace(
        steady,
        trigger=(Trigger.SRC_TENSOR_DONE, Trigger.SUB_DIM_DONE, Trigger.NONE),
        next=(0, 2, 0))
    step = dataclasses.replace(
        steady,
        overrides={**steady.overrides, d: ov},
        trigger=(Trigger.SRC_TENSOR_DONE, Trigger.SUB_DIM_DONE, Trigger.COUNT),
        next=(0, 2, 1), repeat=1)
    out = [ds._assemble(s) for s in (seed, steady2, step)]
    for u in out:
        u.validate(ver)
    return out


def _build_wring_uops(ver):
    """Hand-written uOp program for the spike/reset debt recurrence.
    Stream [pages S, positions N=L1]; position 0 = carry slot:
      j==0 (RSEED/STEP): V = src0 * src1            (seed from stream)
      j>0  (STEADY):     d = src0 - src1; V += src1 * (d >= V)
    out = V at every position. II=4 via 3 bubble slots per element."""
    from concourse.dve_uop import (
        UopConfig, AluOp, AluInp, InpSel, OutSel, OutPath, Trigger, DelayInp,
        ENABLE,
    )
    T_ = Trigger
    N = T_.NONE

    def real(step, trigger, next_uop):
        u = UopConfig()
        u.enable_input(InpSel.SRC_0, 0)
        u.enable_input(InpSel.SRC_1, 1)   # delay lane 0 = C
        u.require_inp0 = ENABLE
        u.require_inp1 = ENABLE
        u.repeat_count = 1
        u.trigger = trigger
        u.next_uop = next_uop
        dp = u.datapath_config
        dp[0].enable_alu(AluOp.MULTIPLY if step else AluOp.SUBTRACT,
                         AluInp.PREV_ALU_OUT, AluInp.PREV_DELAY_0)
        dp[0].pass_through_delay(0)
        dp[1].enable_alu(AluOp.IS_GE, AluInp.PREV_ALU_OUT,
                         AluInp.NEXT_ALU_OUT_A)
        dp[1].pass_through_delay(0)
        dp[1].enable_delay_from_src(DelayInp.PREV_ALU_OUT, 1)
        dp[2].enable_alu(AluOp.MULTIPLY, AluInp.PREV_ALU_OUT,
                         AluInp.PREV_DELAY_0)
        dp[2].pass_through_delay(1)
        if step:
            dp[3].enable_alu(AluOp.BYPASS, AluInp.PREV_DELAY_1,
                             AluInp.PREV_DELAY_1)
        else:
            dp[3].enable_alu(AluOp.ADD, AluInp.CURR_ALU_OUT,
                             AluInp.PREV_ALU_OUT)
        dp[3].alu_out_a_enable = ENABLE
        for s in range(4, 8):
            dp[s].pass_through_alu()
        u.enable_output(OutSel.ALU_OUT, OutPath.WR0_LO)
        return u

    def bub(next_uop, repeat=1):
        u = UopConfig()
        u.repeat_count = repeat
        u.trigger = (T_.COUNT, N, N)
        u.next_uop = next_uop
        dp = u.datapath_config
        dp[2].enable_alu(AluOp.BYPASS, AluInp.NEXT_ALU_OUT_A,
                         AluInp.NEXT_ALU_OUT_A)
        dp[2].alu_out_a_enable = ENABLE
        return u

    return [
        real(True, (T_.COUNT, N, N), (1, 0, 0)),                   # 0 RSEED
        bub((2, 0, 0), repeat=2),                                  # 1 BUBn2
        bub((5, 0, 0)),                                            # 2 BUBn1
        bub((4, 0, 0), repeat=2),                                  # 3 BUBs2
        bub((6, 0, 0)),                                            # 4 BUBs1
        real(False, (T_.SRC_TENSOR_DONE, T_.SUB_DIM_DONE, T_.COUNT),
             (0, 3, 1)),                                           # 5 STEADY
        real(True, (T_.COUNT, N, N), (1, 0, 0)),                   # 6 STEP
    ]


def _register_ops():
    from concourse import dve_ops
    from concourse.dve_ops import DveOp
    from concourse.dve_spec import Spec, Src0, Src1, One, scan, AluOp, lower
    from concourse.dve_uop import DveOpSpec

    def reg(name, spec, segmented=False, uops_fn=None):
        for op in dve_ops.OPS:
            if op.name == name:
                return
        row = max(dve_ops._SUB_OPCODE_FOR_NAME.values()) + 1
        assert row < 0x20
        dve_ops._SUB_OPCODE_FOR_NAME[name] = row
        shas = {}
        specs = {}
        for ver in ("v3", "v4"):
            if uops_fn is not None:
                uops = uops_fn(ver)
            elif segmented:
                uops = _lower_segmented(spec, ver)
            else:
                uops = lower(spec, ver=ver)
            s = DveOpSpec(name=name, opcode=row, uops=uops, rd1_en=True)
            for u in s.uops:
                u.validate(ver)
            shas[ver] = s.sha(ver)
            specs[ver] = s
        op = DveOp(name, spec, subdim=bool(segmented or uops_fn), uops_sha=shas)
        dve_ops.OPS.append(op)
        dve_ops.CUSTOM_DVE_SPECS[name] = spec
        for ver in ("v3", "v4"):
            dve_ops._COMPILE_CACHE[(name, ver)] = specs[ver]

    def _ref_uhseg(in0, in1, s0, s1, imm2):
        c = np.cumsum((in0 * (1.0 + in1)).reshape(in0.shape[0], NPAGES, L),
                      axis=-1, dtype=np.float32)
        return c.reshape(in0.shape)

    def _ref_qgseg(in0, in1, s0, s1, imm2):
        c = np.cumsum(in0.reshape(in0.shape[0], NPAGES, L), axis=-1,
                      dtype=np.float32).reshape(in0.shape)
        return (c * in1).astype(np.float32)

    def _ref_wring(in0, in1, s0, s1, imm2):
        P = in0.shape[0]
        x = in0.reshape(P, NPAGES, L1)
        c = np.asarray(in1).reshape(NPAGES, L1)
        out = np.zeros_like(x)
        for p in range(NPAGES):
            V = x[:, p, 0] * c[p, 0]
            out[:, p, 0] = V
            for j in range(1, L1):
                d = x[:, p, j] - c[p, j]
                V = V + c[p, j] * (d >= V).astype(np.float32)
                out[:, p, j] = V
        return out.reshape(in0.shape)

    reg("UHSEG_ANT", Spec(
        body=scan(AluOp.ADD, Src0 * (One + Src1)),
        reference=_ref_uhseg,
    ), segmented=True)
    reg("QGSEG_ANT", Spec(
        body=scan(AluOp.ADD, Src0) * Src1,
        reference=_ref_qgseg,
    ), segmented=True)
    reg("WRING_ANT", Spec(body=Src0 + Src1, reference=_ref_wring),
        uops_fn=_build_wring_uops)


def _build():
    from contextlib import ExitStack
    import concourse.bass as bass
    import concourse.mybir as mybir
    from concourse.tile import TileContext
    from concourse.dve_ops import get_dve_sub_opcode

    f32 = mybir.dt.float32
    Alu = mybir.AluOpType

    _patch_sim_visit()
    _register_ops()
    ROW_UHSEG = get_dve_sub_opcode("UHSEG_ANT")
    ROW_QGSEG = get_dve_sub_opcode("QGSEG_ANT")
    ROW_WRING = get_dve_sub_opcode("WRING_ANT")

    nc = bass.Bass()
    a_in = nc.dram_tensor("a", (128, NPAGES, T), f32, kind="ExternalInput")
    b_in = nc.dram_tensor("b", (128, NPAGES, T), f32, kind="ExternalInput")
    s_out = nc.dram_tensor("s", (128, NCHUNK, NPAGES, L1), f32,
                           kind="ExternalOutput")

    es = ExitStack()
    # static double-buffered working set (custom-ISA structs need
    # trace-time addresses, so no tile pools here)
    At = es.enter_context(nc.sbuf_tensor([128, 2, NPAGES, L], f32))
    Bt = es.enter_context(nc.sbuf_tensor([128, 2, NPAGES, L], f32))
    qg = es.enter_context(nc.sbuf_tensor([128, 2, NPAGES, L], f32))
    uh = es.enter_context(nc.sbuf_tensor([128, 2, NPAGES, L1], f32))
    w = es.enter_context(nc.sbuf_tensor([128, 2, NPAGES, L1], f32))
    rtile = es.enter_context(nc.sbuf_tensor([128, L], f32))
    rtile32 = es.enter_context(nc.sbuf_tensor([128, NPAGES, L], f32))
    cbuf = es.enter_context(nc.sbuf_tensor([128, NPAGES, L1], f32))
    cmem = es.enter_context(nc.sbuf_tensor([128, L], f32))
    rstage = es.enter_context(nc.sbuf_tensor([128, L], f32))
    qcar = es.enter_context(nc.sbuf_tensor([128, NPAGES], f32))

    mls_addr = {}
    for f in nc.m.functions:
        for a in f.allocations:
            if isinstance(a, mybir.MemoryLocationSet) and a.memorylocations:
                mls_addr[a.memorylocations[0].name] = a.memorylocations[0].addr

    def addr_of(ap):
        return mls_addr[ap.tensor.alloc_name] + ap.offset * mybir.dt.size(ap.dtype)

    def emit_custom(row, out_ap, out_sn, in0_ap, in0_sn, in1_ap, in1_sn, imm0,
                    op1=0):
        """(step, num) given explicitly for each operand's free pattern."""
        def _p2(ap, sn):
            if len(sn) == 4:
                return {"start_addr": {"addr_immediate": addr_of(ap)},
                        "step_elem": [sn[0], sn[2]], "num_elem": [sn[1], sn[3]]}
            return {"start_addr": {"addr_immediate": addr_of(ap)},
                    "step_elem": [sn[0], 0], "num_elem": [sn[1], 1]}

        struct = {
            "src0_mem_pattern": _p2(in0_ap, in0_sn),
            "src1_mem_pattern": {
                "start_addr": {"addr_immediate": addr_of(in1_ap)},
                "step_elem": [in1_sn[0]], "num_elem": [in1_sn[1]]},
            "dst_mem_pattern": _p2(out_ap, out_sn),
            "in0_in1_dtype": {"dtype_lo": 10, "dtype_hi": 10},
            "out_dtype": 10,
            "num_active_channels": 128,
            "imm0_src": 0, "imm1_src": 0, "imm2_src": 1,
            "imm0": {"imm_arith_fp32": float(imm0)},
            "imm1": {"imm_arith_fp32": 0.0},
            "imm2": {"imm_arith_fp32": 0.0},
            "op0": row | (1 << 5),
            "op1": op1,
        }
        return nc.vector.isa(
            nc.isa.Opcode.NEURON_ISA_TPB_OPCODE_CUSTOM_DVE_ANT_0, struct,
            ins=[nc.vector.lower_ap(in0_ap), nc.vector.lower_ap(in1_ap)],
            outs=[nc.vector.lower_ap(out_ap)],
        )

    with TileContext(nc) as tc:
        # rtile[tau] = A_ATT**tau (scan: r' = A_ATT*r, seeded by 1/A_ATT)
        nc.vector.memset(rstage[:], A_ATT)
        nc.vector.tensor_tensor_scan(
            rtile[:], rstage[:], rstage[:], float(1.0 / A_ATT),
            op0=Alu.mult, op1=Alu.bypass)
        for pg in range(NPAGES):
            nc.vector.tensor_copy(rtile32[:, pg, :], rtile[:])
        # cmem[tau] = A_MEM**-tau (scan: r' = r/A_MEM, seeded by A_MEM)
        nc.vector.memset(rstage[:], float(1.0 / A_MEM))
        nc.vector.tensor_tensor_scan(
            cmem[:], rstage[:], rstage[:], float(A_MEM),
            op0=Alu.mult, op1=Alu.bypass)
        # cbuf[pg, 0] = A_MEM**L (carry-slot scale), cbuf[pg, 1+tau] = cmem
        nc.vector.memset(cbuf[:, :, 0], float(A_MEM ** L))
        for pg in range(NPAGES):
            nc.vector.tensor_copy(cbuf[:, pg, 1:], cmem[:])
        nc.vector.memset(qcar[:], 0.0)
        # chunk 0 carry slots: V seeds 0
        nc.vector.memset(uh[:, 0, :, 0], 0.0)
        for ci in range(NCHUNK):
            t0 = ci * L
            k = ci % 2
            kp = (ci - 1) % 2
            nc.sync.dma_start(At[:, k], a_in.ap()[:, :, t0:t0 + L])
            nc.scalar.dma_start(Bt[:, k], b_in.ap()[:, :, t0:t0 + L])

            # attention trace (GAIN-scaled): qg_tau = A_ATT^tau * (kappa +
            # sum_{j<=tau} B_j), B host-prescaled by GAIN*A_ATT^-tau; the
            # cross-chunk carry kappa is injected into the first stream element
            nc.vector.tensor_tensor(
                out=Bt[:, k, :, 0], in0=Bt[:, k, :, 0], in1=qcar[:],
                op=Alu.add)
            emit_custom(ROW_QGSEG,
                        qg[:, k], (1, L, L, NPAGES),
                        Bt[:, k], (1, L, L, NPAGES),
                        rtile32[:], (1, L * NPAGES), 0.0,
                        op1=0x02)
            nc.vector.tensor_scalar(
                out=qcar[:], in0=qg[:, k, :, L - 1], scalar1=float(A_ATT),
                scalar2=None, op0=Alu.mult)

            # chunk-local rescaled membrane: UH = cumsum(A*(1+qg)),
            # written into uh[.., 1:] (carry slot at 0)
            emit_custom(ROW_UHSEG,
                        uh[:, k, :, 1:], (1, L, L1, NPAGES),
                        At[:, k], (1, L, L, NPAGES),
                        qg[:, k], (1, L * NPAGES), 0.0,
                        op1=0x02)

            # carry slot for the NEXT chunk's V seed: raw carry =
            # V_L - UH_L of THIS chunk (scaled by alpha^L inside WRING)
            if ci > 0:
                nc.vector.tensor_tensor(
                    out=uh[:, k, :, 0], in0=w[:, kp, :, L], in1=uh[:, kp, :, L],
                    op=Alu.subtract)

            # spike recurrence: one WRING instruction over [NPAGES, L1]
            emit_custom(ROW_WRING,
                        w[:, k], (1, L1, L1, NPAGES),
                        uh[:, k], (1, L1, L1, NPAGES),
                        cbuf[:], (1, NPAGES * L1), 0.0,
                        op1=0x02)

            nc.scalar.dma_start(s_out.ap()[:, ci], w[:, k])
    es.close()
    nc.m.ant_custom_dve_ops = sorted(
        {*nc.m.ant_custom_dve_ops, "UHSEG_ANT", "QGSEG_ANT", "WRING_ANT"})
    _split_waits(nc)
    return nc


def kernel(x: np.ndarray, attention_weights: np.ndarray) -> np.ndarray:
    from concourse.bass_utils import run_bass_kernel_spmd

    if "nc" not in _STATE:
        _STATE["nc"] = _build()
    nc = _STATE["nc"]

    x = np.ascontiguousarray(x, dtype=np.float32)
    aw = np.ascontiguousarray(attention_weights, dtype=np.float32)

    # host layout + prescale: [b, t, j, c] -> [c, b, j, t]; chunk-local
    # rescale alpha^-(t mod L) for both streams
    invm = np.exp((np.arange(T, dtype=np.float64) % L) / 20.0).astype(np.float32)
    A_all = np.ascontiguousarray(
        x.reshape(B, T, D // 128, 128).transpose(3, 0, 2, 1))
    A_all *= invm[None, None, None, :]
    B_all = np.ascontiguousarray(
        aw.reshape(B, T, D // 128, 128).transpose(3, 0, 2, 1))
    invb = (GAIN * np.exp((np.arange(T, dtype=np.float64) % L) / 50.0)
            ).astype(np.float32)
    B_all *= invb[None, None, None, :]

    in_maps = [
        {"a": A_all[:, k * BPC:(k + 1) * BPC].reshape(128, NPAGES, T),
         "b": B_all[:, k * BPC:(k + 1) * BPC].reshape(128, NPAGES, T)}
        for k in range(NCORES)
    ]
    res = run_bass_kernel_spmd(nc, in_maps, core_ids=list(range(NCORES)))

    out = np.empty((B, T, D), dtype=np.float32)
    for k in range(NCORES):
        wtr = np.asarray(res.results[k]["s"], dtype=np.float32)
        # V stream [c, ci, pg, j]; spike where V moved (j=0 is carry slot)
        s = (wtr[:, :, :, 1:] != wtr[:, :, :, :-1]).astype(np.float32)
        # [c, ci, pg=(b_local, j128), tau] -> [b, t, d]
        s = s.reshape(128, NCHUNK, BPC, D // 128, L).transpose(2, 1, 4, 3, 0)
        out[k * BPC:(k + 1) * BPC] = s.reshape(BPC, T, D)
    return out


# revision 10
# speedup vs baseline: 2.0769x; 1.4714x over previous
"""Trainium2 Bass kernel for AttentionOptimizedNeuron (v3 — WRING scan).

Model (per channel c=(b,d), recurrence over t):
    att = A_ATT*att + aw[t]*GAIN
    mem = A_MEM*mem + x[t]*(1+att)
    s   = (mem >= 1);  mem -= s          (subtract reset)

v3 vs v2: the spike/reset recurrence runs as ONE hand-written custom-DVE
scan instruction per chunk (WRING_ANT) instead of 2048 tiny chained
instructions. In chunk-rescaled debt space (V = cumulative reset debt,
UH = chunk-local cumsum of the rescaled drive, C_tau = A_MEM^-tau):

    s_tau = (UH_tau - C_tau >= V);  V += C_tau * s_tau

The WRING uOp program closes this feedback loop on the DVE datapath:
stage0 d=UH-C, stage1 cmp=IS_GE(d, V via NEXT_ALU_OUT_A), stage2
gate=cmp*C, stage3 V+=gate (CURR feedback + a-flop). V is brigaded
backward (stage3.a -> stage2.a) by bubble slots; II=4 cycles/element.
Per-page V re-seed rides the stream: each page's position 0 holds the
carry (V_L - UH_L of the previous chunk), consumed by a STEP uop state
(V <- carry * alpha^L via the C-stream boundary slot), triggered by
SUB_DIM_DONE at page wrap. Spikes recovered on host as diff(V) != 0.

Host prescale is chunk-local (alpha^-(t mod L)), so magnitudes stay in
[1, 602] per chunk regardless of t.
"""

import numpy as np

A_MEM = float(np.exp(-1.0 / 20.0))
A_ATT = float(np.exp(-1.0 / 50.0))
GAIN = 0.2

B, T, D = 32, 1024, 1024
NCORES = 8
BPC = B // NCORES            # batches per core
NPAGES = BPC * (D // 128)    # 32 channel pages of 128
L = 128                      # time steps per chunk (SBUF sizing)
NCHUNK = T // L
L1 = L + 1                   # page stream length incl. carry slot
NPAIR = NPAGES // 2          # WRING pairs: page pg -> (p=pg%16, h=pg//16)

_STATE = {}


def _split_waits(nc):
    """walrus CoreV3 in this container rejects >1 sync wait per instruction.
    Tile attaches several; move the extras onto same-engine nops inserted
    immediately before the instruction (identical blocking semantics)."""
    from concourse import mybir

    for f in nc.m.functions:
        for blk in f.blocks:
            new_insts = []
            for inst in blk.instructions:
                si = getattr(inst, "sync_info", None)
                if si is not None and si.on_wait and len(si.on_wait) > 1:
                    waits = list(si.on_wait)
                    si.on_wait = waits[-1:]
                    for w in waits[:-1]:
                        nop = mybir.InstNoOp(
                            name=nc.get_next_instruction_name(),
                            opcode="NoOp",
                            engine=inst.engine,
                            sync_info=mybir.SyncInfo(on_wait=[w], on_update=[]),
                        )
                        new_insts.append(nop)
                new_insts.append(inst)
            if len(new_insts) != len(blk.instructions):
                blk.instructions[:] = new_insts


def _patch_sim_visit():
    import concourse.bass_interp as bi
    if hasattr(bi, "_orig_visit_instisa"):
        return
    bi._orig_visit_instisa = bi._visit_InstISA

    def _pv(isa, instruction, core_sim):
        if instruction.isa_opcode in (0xAE, 0xAF, 0xEE, 0xEF):
            return
        return bi._orig_visit_instisa(isa, instruction, core_sim)

    bi._visit_InstISA = _pv


def _lower_segmented(spec, ver):
    """lower() for a single-scan Spec, with a per-subdim STEP state so the
    scan accumulator re-seeds at each SUB_DIM boundary (page)."""
    import dataclasses
    from concourse import dve_spec as ds
    from concourse.dve_uop import Trigger

    ds._validate_body(spec, ver)
    spec2 = ds._hoist_stream_invariant_ops(spec)
    scans = ds._collect(spec2.body, ds.Scan)
    latches = ds._collect(spec2.body, ds.Latch)
    assert len(scans) == 1 and not latches
    p = ds._build_placement(spec2, scans, ds.N_STAGES[ver], ds.N_LANES[ver])
    states = ds._build_state_machine(spec2, scans, latches, p)
    assert len(states) == 2, "expected [seed, steady]"
    seed, steady = states
    sc = scans[0]
    d = p.node_stage[sc]
    if isinstance(sc.expr, ds.Alu):
        ov = ds._Stage(ds.AluOp.BYPASS, ds.PREV)
    else:
        ov = ds._Stage(ds.AluOp.BYPASS, sc.expr)
    steady2 = dataclasses.replace(
        steady,
        trigger=(Trigger.SRC_TENSOR_DONE, Trigger.SUB_DIM_DONE, Trigger.NONE),
        next=(0, 2, 0))
    step = dataclasses.replace(
        steady,
        overrides={**steady.overrides, d: ov},
        trigger=(Trigger.SRC_TENSOR_DONE, Trigger.SUB_DIM_DONE, Trigger.COUNT),
        next=(0, 2, 1), repeat=1)
    out = [ds._assemble(s) for s in (seed, steady2, step)]
    for u in out:
        u.validate(ver)
    return out


def _build_wring_uops(ver):
    """Hand-written 2-chain uOp program for the spike/reset debt recurrence.
    Stream [pairs S, positions N=2*L1], element (p, j, h) at p*2*L1+2j+h;
    chain A (h=0) on stages 0-3, chain B (h=1) on stages 4-7. Per chain:
      j==0 (STEP): V = src0 * src1             (seed from stream)
      j>0:         d = src0 - src1; V += src1 * (d >= V)
    out = V. 2 bubble slots per pair brigade V backward (stage3.a->stage2.a
    and stage6.a->stage5.a); 2 cycles/element."""
    from concourse.dve_uop import (
        UopConfig, AluOp, AluInp, InpSel, OutSel, OutPath, Trigger, DelayInp,
        ENABLE,
    )
    T_ = Trigger
    N = T_.NONE

    def dp_a(dp, step):
        # chain A: d@0, cmp@1 (reads stage2.a), gate@2, V@3(+a-flop);
        # V captured to lane2 at st4, emitted via DELAY_2
        dp[0].enable_alu(AluOp.MULTIPLY if step else AluOp.SUBTRACT,
                         AluInp.PREV_ALU_OUT, AluInp.PREV_DELAY_0)
        dp[0].pass_through_delay(0)
        dp[1].enable_alu(AluOp.IS_GE, AluInp.PREV_ALU_OUT,
                         AluInp.NEXT_ALU_OUT_A)
        dp[1].pass_through_delay(0)
        dp[1].enable_delay_from_src(DelayInp.PREV_ALU_OUT, 1)
        dp[2].enable_alu(AluOp.MULTIPLY, AluInp.PREV_ALU_OUT,
                         AluInp.PREV_DELAY_0)
        dp[2].pass_through_delay(1)
        if step:
            dp[3].enable_alu(AluOp.BYPASS, AluInp.PREV_DELAY_1,
                             AluInp.PREV_DELAY_1)
        else:
            dp[3].enable_alu(AluOp.ADD, AluInp.CURR_ALU_OUT,
                             AluInp.PREV_ALU_OUT)
        dp[3].alu_out_a_enable = ENABLE
        dp[4].enable_delay_from_src(DelayInp.PREV_ALU_OUT, 2)
        dp[5].pass_through_delay(2)
        dp[6].pass_through_delay(2)
        dp[7].pass_through_delay(2)

    def dp_b(dp, step):
        # chain B: d@0 -> lane1, cmp@4 (reads stage5.a), gate@5, V@6(+a),
        # out@7 via ALU_OUT
        dp[0].enable_alu(AluOp.MULTIPLY if step else AluOp.SUBTRACT,
                         AluInp.PREV_ALU_OUT, AluInp.PREV_DELAY_0)
        dp[0].pass_through_delay(0)
        dp[1].enable_delay_from_src(DelayInp.PREV_ALU_OUT, 1)
        dp[1].pass_through_delay(0)
        dp[1].alu_out_enable = 0
        dp[2].pass_through_delay(0, 1)
        dp[3].pass_through_delay(0, 1)
        dp[4].enable_alu(AluOp.IS_GE, AluInp.PREV_DELAY_1,
                         AluInp.NEXT_ALU_OUT_A)
        dp[4].pass_through_delay(0, 1)
        dp[5].enable_alu(AluOp.MULTIPLY, AluInp.PREV_ALU_OUT,
                         AluInp.PREV_DELAY_0)
        dp[5].pass_through_delay(1)
        if step:
            dp[6].enable_alu(AluOp.BYPASS, AluInp.PREV_DELAY_1,
                             AluInp.PREV_DELAY_1)
        else:
            dp[6].enable_alu(AluOp.ADD, AluInp.CURR_ALU_OUT,
                             AluInp.PREV_ALU_OUT)
        dp[6].alu_out_a_enable = ENABLE
        dp[7].enable_alu(AluOp.BYPASS, AluInp.PREV_ALU_OUT,
                         AluInp.PREV_ALU_OUT)

    def real2(chain, step, trigger, next_uop):
        u = UopConfig()
        u.enable_input(InpSel.SRC_0, 0)
        u.enable_input(InpSel.SRC_1, 1)
        u.require_inp0 = ENABLE
        u.require_inp1 = ENABLE
        u.repeat_count = 1
        u.trigger = trigger
        u.next_uop = next_uop
        if chain == "a":
            dp_a(u.datapath_config, step)
            u.enable_output(OutSel.DELAY_2, OutPath.WR0_LO)
        else:
            dp_b(u.datapath_config, step)
            u.enable_output(OutSel.ALU_OUT, OutPath.WR0_LO)
            u.accum_enabled = ENABLE
        return u

    def bub2(next_uop, repeat=1):
        u = UopConfig()
        u.repeat_count = repeat
        u.trigger = (T_.COUNT, N, N)
        u.next_uop = next_uop
        dp = u.datapath_config
        dp[2].enable_alu(AluOp.BYPASS, AluInp.NEXT_ALU_OUT_A,
                         AluInp.NEXT_ALU_OUT_A)
        dp[2].alu_out_a_enable = ENABLE
        dp[5].enable_alu(AluOp.BYPASS, AluInp.NEXT_ALU_OUT_A,
                         AluInp.NEXT_ALU_OUT_A)
        dp[5].alu_out_a_enable = ENABLE
        return u

    return [
        real2("a", True, (T_.COUNT, N, N), (1, 0, 0)),                 # 0
        real2("b", True, (T_.COUNT, N, N), (2, 0, 0)),                 # 1
        bub2((3, 0, 0), repeat=2),                                     # 2
        real2("a", False, (T_.SRC_TENSOR_DONE, T_.COUNT, N), (0, 4, 0)),  # 3
        real2("b", False, (T_.SRC_TENSOR_DONE, T_.SUB_DIM_DONE, T_.COUNT),
              (0, 5, 2)),                                              # 4
        bub2((6, 0, 0), repeat=2),                                     # 5
        real2("a", True, (T_.COUNT, N, N), (1, 0, 0)),                 # 6
    ]


def _register_ops():
    from concourse import dve_ops
    from concourse.dve_ops import DveOp
    from concourse.dve_spec import Spec, Src0, Src1, One, scan, AluOp, lower
    from concourse.dve_uop import DveOpSpec

    def reg(name, spec, segmented=False, uops_fn=None):
        for op in dve_ops.OPS:
            if op.name == name:
                return
        row = max(dve_ops._SUB_OPCODE_FOR_NAME.values()) + 1
        assert row < 0x20
        dve_ops._SUB_OPCODE_FOR_NAME[name] = row
        shas = {}
        specs = {}
        for ver in ("v3", "v4"):
            if uops_fn is not None:
                uops = uops_fn(ver)
            elif segmented:
                uops = _lower_segmented(spec, ver)
            else:
                uops = lower(spec, ver=ver)
            s = DveOpSpec(name=name, opcode=row, uops=uops, rd1_en=True)
            for u in s.uops:
                u.validate(ver)
            shas[ver] = s.sha(ver)
            specs[ver] = s
        op = DveOp(name, spec, subdim=bool(segmented or uops_fn), uops_sha=shas)
        dve_ops.OPS.append(op)
        dve_ops.CUSTOM_DVE_SPECS[name] = spec
        for ver in ("v3", "v4"):
            dve_ops._COMPILE_CACHE[(name, ver)] = specs[ver]

    def _ref_uhseg(in0, in1, s0, s1, imm2):
        c = np.cumsum((in0 * (1.0 + in1)).reshape(in0.shape[0], NPAGES, L),
                      axis=-1, dtype=np.float32)
        return c.reshape(in0.shape)

    def _ref_qgseg(in0, in1, s0, s1, imm2):
        c = np.cumsum(in0.reshape(in0.shape[0], NPAGES, L), axis=-1,
                      dtype=np.float32).reshape(in0.shape)
        return (c * in1).astype(np.float32)

    def _ref_wring(in0, in1, s0, s1, imm2):
        P = in0.shape[0]
        x = in0.reshape(P, NPAIR, L1, 2)
        c = np.asarray(in1).reshape(NPAIR, L1, 2)
        out = np.zeros_like(x)
        for p in range(NPAIR):
            for h in range(2):
                V = x[:, p, 0, h] * c[p, 0, h]
                out[:, p, 0, h] = V
                for j in range(1, L1):
                    d = x[:, p, j, h] - c[p, j, h]
                    V = V + c[p, j, h] * (d >= V).astype(np.float32)
                    out[:, p, j, h] = V
        return out.reshape(in0.shape)

    reg("UHSEG_ANT", Spec(
        body=scan(AluOp.ADD, Src0 * (One + Src1)),
        reference=_ref_uhseg,
    ), segmented=True)
    reg("QGSEG_ANT", Spec(
        body=scan(AluOp.ADD, Src0) * Src1,
        reference=_ref_qgseg,
    ), segmented=True)
    reg("WRING_ANT", Spec(body=Src0 + Src1, reference=_ref_wring),
        uops_fn=_build_wring_uops)


def _build():
    from contextlib import ExitStack
    import concourse.bass as bass
    import concourse.mybir as mybir
    from concourse.tile import TileContext
    from concourse.dve_ops import get_dve_sub_opcode

    f32 = mybir.dt.float32
    Alu = mybir.AluOpType

    _patch_sim_visit()
    _register_ops()
    ROW_UHSEG = get_dve_sub_opcode("UHSEG_ANT")
    ROW_QGSEG = get_dve_sub_opcode("QGSEG_ANT")
    ROW_WRING = get_dve_sub_opcode("WRING_ANT")

    nc = bass.Bass()
    a_in = nc.dram_tensor("a", (128, NPAGES, T), f32, kind="ExternalInput")
    b_in = nc.dram_tensor("b", (128, NPAGES, T), f32, kind="ExternalInput")
    s_out = nc.dram_tensor("s", (128, NCHUNK, NPAIR, L1, 2), f32,
                           kind="ExternalOutput")

    es = ExitStack()
    # static double-buffered working set (custom-ISA structs need
    # trace-time addresses, so no tile pools here)
    At = es.enter_context(nc.sbuf_tensor([128, 2, NPAGES, L], f32))
    Bt = es.enter_context(nc.sbuf_tensor([128, 2, NPAGES, L], f32))
    qg = es.enter_context(nc.sbuf_tensor([128, 2, NPAGES, L], f32))
    uh = es.enter_context(nc.sbuf_tensor([128, 2, NPAIR, L1, 2], f32))
    w = es.enter_context(nc.sbuf_tensor([128, 2, NPAIR, L1, 2], f32))
    rtile = es.enter_context(nc.sbuf_tensor([128, L], f32))
    rtile32 = es.enter_context(nc.sbuf_tensor([128, NPAGES, L], f32))
    cbuf = es.enter_context(nc.sbuf_tensor([128, NPAIR, L1, 2], f32))
    cmem = es.enter_context(nc.sbuf_tensor([128, L], f32))
    rstage = es.enter_context(nc.sbuf_tensor([128, L], f32))
    qcar = es.enter_context(nc.sbuf_tensor([128, NPAGES], f32))

    mls_addr = {}
    for f in nc.m.functions:
        for a in f.allocations:
            if isinstance(a, mybir.MemoryLocationSet) and a.memorylocations:
                mls_addr[a.memorylocations[0].name] = a.memorylocations[0].addr

    def addr_of(ap):
        return mls_addr[ap.tensor.alloc_name] + ap.offset * mybir.dt.size(ap.dtype)

    def emit_custom(row, out_ap, out_sn, in0_ap, in0_sn, in1_ap, in1_sn, imm0,
                    op1=0):
        """(step, num) given explicitly for each operand's free pattern."""
        def _p2(ap, sn):
            if len(sn) == 4:
                return {"start_addr": {"addr_immediate": addr_of(ap)},
                        "step_elem": [sn[0], sn[2]], "num_elem": [sn[1], sn[3]]}
            return {"start_addr": {"addr_immediate": addr_of(ap)},
                    "step_elem": [sn[0], 0], "num_elem": [sn[1], 1]}

        struct = {
            "src0_mem_pattern": _p2(in0_ap, in0_sn),
            "src1_mem_pattern": {
                "start_addr": {"addr_immediate": addr_of(in1_ap)},
                "step_elem": [in1_sn[0]], "num_elem": [in1_sn[1]]},
            "dst_mem_pattern": _p2(out_ap, out_sn),
            "in0_in1_dtype": {"dtype_lo": 10, "dtype_hi": 10},
            "out_dtype": 10,
            "num_active_channels": 128,
            "imm0_src": 0, "imm1_src": 0, "imm2_src": 1,
            "imm0": {"imm_arith_fp32": float(imm0)},
            "imm1": {"imm_arith_fp32": 0.0},
            "imm2": {"imm_arith_fp32": 0.0},
            "op0": row | (1 << 5),
            "op1": op1,
        }
        return nc.vector.isa(
            nc.isa.Opcode.NEURON_ISA_TPB_OPCODE_CUSTOM_DVE_ANT_0, struct,
            ins=[nc.vector.lower_ap(in0_ap), nc.vector.lower_ap(in1_ap)],
            outs=[nc.vector.lower_ap(out_ap)],
        )

    with TileContext(nc) as tc:
        # rtile[tau] = A_ATT**tau (scan: r' = A_ATT*r, seeded by 1/A_ATT)
        nc.vector.memset(rstage[:], A_ATT)
        nc.vector.tensor_tensor_scan(
            rtile[:], rstage[:], rstage[:], float(1.0 / A_ATT),
            op0=Alu.mult, op1=Alu.bypass)
        for pg in range(NPAGES):
            nc.vector.tensor_copy(rtile32[:, pg, :], rtile[:])
        # cmem[tau] = A_MEM**-tau (scan: r' = r/A_MEM, seeded by A_MEM)
        nc.vector.memset(rstage[:], float(1.0 / A_MEM))
        nc.vector.tensor_tensor_scan(
            cmem[:], rstage[:], rstage[:], float(A_MEM),
            op0=Alu.mult, op1=Alu.bypass)
        # cbuf[p, 0, h] = A_MEM**L (carry-slot scale), cbuf[p, 1+tau, h] = cmem
        nc.vector.memset(cbuf[:, :, 0, :], float(A_MEM ** L))
        for p in range(NPAIR):
            for h in range(2):
                nc.vector.tensor_copy(cbuf[:, p, 1:, h], cmem[:])
        nc.vector.memset(qcar[:], 0.0)
        # chunk 0 carry slots: V seeds 0
        nc.vector.memset(uh[:, 0, :, 0, :], 0.0)
        for ci in range(NCHUNK):
            t0 = ci * L
            k = ci % 2
            kp = (ci - 1) % 2
            nc.sync.dma_start(At[:, k], a_in.ap()[:, :, t0:t0 + L])
            nc.scalar.dma_start(Bt[:, k], b_in.ap()[:, :, t0:t0 + L])

            # attention trace (GAIN-scaled): qg_tau = A_ATT^tau * (kappa +
            # sum_{j<=tau} B_j), B host-prescaled by GAIN*A_ATT^-tau; the
            # cross-chunk carry kappa is injected into the first stream element
            nc.vector.tensor_tensor(
                out=Bt[:, k, :, 0], in0=Bt[:, k, :, 0], in1=qcar[:],
                op=Alu.add)
            emit_custom(ROW_QGSEG,
                        qg[:, k], (1, L, L, NPAGES),
                        Bt[:, k], (1, L, L, NPAGES),
                        rtile32[:], (1, L * NPAGES), 0.0,
                        op1=0x02)
            nc.vector.tensor_scalar(
                out=qcar[:], in0=qg[:, k, :, L - 1], scalar1=float(A_ATT),
                scalar2=None, op0=Alu.mult)

            # chunk-local rescaled membrane: UH = cumsum(A*(1+qg)), written
            # interleaved: page pg=(p,h) -> uh[p, 1+tau, h]; two calls (one
            # per half) since the h-offset breaks a single 2D pattern
            for h in range(2):
                pg0 = h * NPAIR
                emit_custom(ROW_UHSEG,
                            uh[:, k, :, 1:, h], (2, L, 2 * L1, NPAIR),
                            At[:, k, pg0:pg0 + NPAIR], (1, L, L, NPAIR),
                            qg[:, k, pg0:pg0 + NPAIR], (1, L * NPAIR), 0.0,
                            op1=0x02)

            # carry slot for the NEXT chunk's V seed: raw carry =
            # V_L - UH_L of THIS chunk (scaled by alpha^L inside WRING)
            if ci > 0:
                nc.vector.tensor_tensor(
                    out=uh[:, k, :, 0, :], in0=w[:, kp, :, L, :],
                    in1=uh[:, kp, :, L, :], op=Alu.subtract)

            # spike recurrence: one 2-chain WRING over [NPAIR, 2*L1]
            emit_custom(ROW_WRING,
                        w[:, k], (1, 2 * L1, 2 * L1, NPAIR),
                        uh[:, k], (1, 2 * L1, 2 * L1, NPAIR),
                        cbuf[:], (1, NPAIR * L1 * 2), 0.0,
                        op1=0x02)

            nc.scalar.dma_start(s_out.ap()[:, ci], w[:, k])
    es.close()
    nc.m.ant_custom_dve_ops = sorted(
        {*nc.m.ant_custom_dve_ops, "UHSEG_ANT", "QGSEG_ANT", "WRING_ANT"})
    _split_waits(nc)
    return nc


def kernel(x: np.ndarray, attention_weights: np.ndarray) -> np.ndarray:
    from concourse.bass_utils import run_bass_kernel_spmd

    if "nc" not in _STATE:
        _STATE["nc"] = _build()
    nc = _STATE["nc"]

    x = np.ascontiguousarray(x, dtype=np.float32)
    aw = np.ascontiguousarray(attention_weights, dtype=np.float32)

    # host layout + prescale: [b, t, j, c] -> [c, b, j, t]; chunk-local
    # rescale alpha^-(t mod L) for both streams
    invm = np.exp((np.arange(T, dtype=np.float64) % L) / 20.0).astype(np.float32)
    A_all = np.ascontiguousarray(
        x.reshape(B, T, D // 128, 128).transpose(3, 0, 2, 1))
    A_all *= invm[None, None, None, :]
    B_all = np.ascontiguousarray(
        aw.reshape(B, T, D // 128, 128).transpose(3, 0, 2, 1))
    invb = (GAIN * np.exp((np.arange(T, dtype=np.float64) % L) / 50.0)
            ).astype(np.float32)
    B_all *= invb[None, None, None, :]

    in_maps = [
        {"a": A_all[:, k * BPC:(k + 1) * BPC].reshape(128, NPAGES, T),
         "b": B_all[:, k * BPC:(k + 1) * BPC].reshape(128, NPAGES, T)}
        for k in range(NCORES)
    ]
    res = run_bass_kernel_spmd(nc, in_maps, core_ids=list(range(NCORES)))

    out = np.empty((B, T, D), dtype=np.float32)
    for k in range(NCORES):
        wtr = np.asarray(res.results[k]["s"], dtype=np.float32)
        # V stream [c, ci, p, j, h]; spike where V moved (j=0 is carry slot)
        s = (wtr[:, :, :, 1:, :] != wtr[:, :, :, :-1, :]).astype(np.float32)
        # [c, ci, p, tau, h] -> [c, ci, pg=16h+p, tau] -> [b, t, d]
        s = np.moveaxis(s, 4, 2).reshape(128, NCHUNK, NPAGES, L)
        s = s.reshape(128, NCHUNK, BPC, D // 128, L).transpose(2, 1, 4, 3, 0)
        out[k * BPC:(k + 1) * BPC] = s.reshape(BPC, T, D)
    return out
